# revision 22
# baseline (speedup 1.0000x reference)
"""Trainium2 Bass kernel for GQA attention block (B=2, S=2048, HS=2048, H=16, HKV=4, D=128).

Strategy (8 NeuronCores, SPMD):
  - Head-parallel: core c computes q-heads {2c, 2c+1} and kv-head c//2 for BOTH batches.
  - Fused QKV projection: one 512-wide rhs stream [q0|q1|k|v] per contraction tile.
  - Per-head RMS norm folded into RoPE via scalar_tensor_tensor (per-token rinv is a
    per-partition scalar in [tok, d] layout); norm stats via one scalar Square + one
    vector segmented reduce; k-RoPE on GpSimd to balance engines.
  - q/k transposed to [d, tok] via PE transpose.
  - Causal flash attention in transposed layout: S^T = K_rope @ Q_rope^T ([kv, q]),
    scores exp'd in PAIRS (one ACT instr over a 2-bank PSUM AP) with 1/sqrt(D) folded
    into the exp scale; binary causal mask by multiply on diagonal pairs;
    O^T = V^T @ P^T accumulated in PSUM; softmax denominators via ones-vector matmul
    on vector-summed PAIRS of P tiles (halves the PE cost of denominators).
  - One 8-rank AllToAll per head redistributes head-shards -> (batch, seq-strip)
    shards; head-0's AllToAll overlaps head-1 attention, head-1's overlaps the
    head-0 half of the output projection.
  - Output projection per strip; host concatenates the 8 strips.
"""

import sys

sys.path.insert(0, "/opt/trn_rl_repo")

import numpy as np
import ml_dtypes

BF16 = ml_dtypes.bfloat16

B, H, HKV, D = 2, 16, 4, 128
EPS = 1e-6
P = 128
N_CORES = 8


def build(S=2048, HS=2048):
    """Build + compile the SPMD graph. Returns the Bacc module."""
    import concourse.bacc as bacc
    import concourse.tile as tile
    import concourse.mybir as mybir

    dt = mybir.dt
    f32 = dt.float32
    bf16 = dt.bfloat16
    AF = mybir.ActivationFunctionType
    ALU = mybir.AluOpType

    T = S // P          # tok tiles per batch
    M = 2 * T           # tok tiles total (2 batches)
    KT = HS // P        # contraction tiles for qkv projection
    KO = (H * D) // P   # contraction tiles for o projection (16)
    CW = S // 4         # q-chunk width == strip width
    CB = CW // P        # kv blocks per chunk step
    OCH = HS // 512     # output column chunks
    NQ = 2              # q heads per core
    SCL = float(D) ** -0.5

    nc = bacc.Bacc("TRN2", target_bir_lowering=False, debug=False,
                   enable_asserts=True, num_devices=N_CORES)

    xT = nc.dram_tensor("xT", [M, P, HS], bf16, kind="ExternalInput")
    wqkvT = nc.dram_tensor("wqkvT", [P, KT * 512], bf16, kind="ExternalInput")
    woT = nc.dram_tensor("woT", [P, KO * HS], bf16, kind="ExternalInput")
    cosq_d = nc.dram_tensor("cosq", [P, T * D], bf16, kind="ExternalInput")
    sinq_d = nc.dram_tensor("sinq", [P, T * D], bf16, kind="ExternalInput")
    cosk_d = nc.dram_tensor("cosk", [P, T * D], bf16, kind="ExternalInput")
    sink_d = nc.dram_tensor("sink", [P, T * D], bf16, kind="ExternalInput")
    masks_d = nc.dram_tensor("masks", [P, CB, CW], bf16, kind="ExternalInput")
    onesq_d = nc.dram_tensor("onesq", [P, P], bf16, kind="ExternalInput")
    ident_d = nc.dram_tensor("ident", [P, P], bf16, kind="ExternalInput")
    out_d = nc.dram_tensor("out", [CW, HS], f32, kind="ExternalOutput")

    with tile.TileContext(nc) as tc:
        with tc.tile_pool(name="const", bufs=1) as cpool, \
             tc.tile_pool(name="weights", bufs=1) as wpool, \
             tc.tile_pool(name="qkv", bufs=1) as qkvpool, \
             tc.tile_pool(name="xin", bufs=6) as xin, \
             tc.tile_pool(name="dram", bufs=1, space="DRAM") as dpool:

            # DMA FIFO order matters: the first matmul needs only wqkv
            # chunk 0 + xm0, so those two go first.
            wqkv_sb = wpool.tile([P, KT, 512], bf16, name="wqkv_sb")
            wqkv_src = wqkvT.ap().rearrange("p (k f) -> p k f", k=KT)
            nc.sync.dma_start(wqkv_sb[:, 0:4, :], wqkv_src[:, 0:4, :])

            xms = {}

            def load_xm(m):
                t_ = xin.tile([P, KT, P], bf16, tag="xm", name=f"xm{m}")
                nc.sync.dma_start(t_[:], xT.ap()[m].rearrange("p (k t) -> p k t", k=KT))
                xms[m] = t_

            load_xm(0)
            nc.sync.dma_start(wqkv_sb[:, 4:8, :], wqkv_src[:, 4:8, :])
            load_xm(1)
            nc.sync.dma_start(wqkv_sb[:, 8:12, :], wqkv_src[:, 8:12, :])
            load_xm(2)
            nc.sync.dma_start(wqkv_sb[:, 12:16, :], wqkv_src[:, 12:16, :])
            load_xm(3)

            cosq_sb = cpool.tile([P, T, D], bf16, name="cosq_sb")
            sinq_sb = cpool.tile([P, T, D], bf16, name="sinq_sb")
            cosk_sb = cpool.tile([P, T, D], bf16, name="cosk_sb")
            sink_sb = cpool.tile([P, T, D], bf16, name="sink_sb")
            nc.sync.dma_start(cosq_sb[:], cosq_d.ap().rearrange("p (t d) -> p t d", t=T))
            nc.sync.dma_start(sinq_sb[:], sinq_d.ap().rearrange("p (t d) -> p t d", t=T))
            nc.sync.dma_start(cosk_sb[:], cosk_d.ap().rearrange("p (t d) -> p t d", t=T))
            nc.sync.dma_start(sink_sb[:], sink_d.ap().rearrange("p (t d) -> p t d", t=T))

            masks_sb = cpool.tile([P, CB, CW], bf16, name="masks_sb")
            nc.sync.dma_start(masks_sb[:], masks_d.ap())
            onesq_sb = cpool.tile([P, P], bf16, name="onesq_sb")
            nc.sync.dma_start(onesq_sb[:], onesq_d.ap())
            ident_sb = cpool.tile([P, P], bf16, name="ident_sb")
            nc.sync.dma_start(ident_sb[:], ident_d.ap())
            eps_sb = cpool.tile([P, 1], f32, name="eps_sb")
            nc.gpsimd.memset(eps_sb[:], EPS)

            qT_sb = qkvpool.tile([P, NQ, 2 * S], bf16, name="qT_sb")
            kT_sb = qkvpool.tile([P, 2 * S], bf16, name="kT_sb")
            v_sb = qkvpool.tile([P, M, D], bf16, name="v_sb")

            a2a_in = [dpool.tile([1024, CW], bf16, name=f"a2a_in{h}")
                      for h in range(NQ)]
            a2a_out = [dpool.tile([1024, CW], bf16, name=f"a2a_out{h}")
                       for h in range(NQ)]
            warm_in = dpool.tile([1024, 64], bf16, name="warm_in")
            warm_out = dpool.tile([1024, 64], bf16, name="warm_out")
            # warmup AllToAll: absorbs the ~11us first-collective start delay
            nc.gpsimd.collective_compute(
                "AllToAll", mybir.AluOpType.bypass,
                ins=[warm_in[:].opt()], outs=[warm_out[:].opt()],
                replica_groups=[list(range(N_CORES))],
            )

            # ---------------- stage 1+2: QKV projection, RMS norm + RoPE (fused),
            # DMA-XBAR transpose to [d, tok]
            with tc.tile_pool(name="s12", bufs=4) as s12, \
                 tc.tile_pool(name="ps12", bufs=2, space="PSUM") as ps12:
                tp_pend = []

                def emit_tps(keep=0):
                    while len(tp_pend) > keep:
                        srct, dst = tp_pend.pop(0)
                        tp = ps12.tile([P, P], bf16, tag="tp", bufs=3)
                        nc.tensor.transpose(tp, srct, ident_sb)
                        nc.scalar.copy(dst, tp)

                for m in range(M):
                    b, mm = m // T, m % T
                    if m + 4 < M:
                        load_xm(m + 4)
                    xm = xms.pop(m)
                    ps_qkv = ps12.tile([P, 512], f32, tag="qkv")
                    for k in range(KT):
                        nc.tensor.matmul(ps_qkv, xm[:, k, :], wqkv_sb[:, k, :],
                                         start=(k == 0), stop=(k == KT - 1))
                        if k == 7:
                            # transposes deferred TWO m-cycles: their ro/rok
                            # are long finished, so they slot in with zero wait
                            emit_tps(keep=3 if m < M - 1 else 0)
                    # scalar drains PSUM quickly: qkv copy + v copy only
                    qks = s12.tile([P, 3, P], bf16, tag="qks")
                    nc.scalar.copy(qks.rearrange("p a b -> p (a b)"),
                                   ps_qkv[:, 0:384])
                    nc.scalar.copy(v_sb[:, m, :], ps_qkv[:, 384:512])
                    # norm stats on vector (from the bf16 copy)
                    sq = s12.tile([P, 3, P], bf16, tag="sq")
                    nc.vector.tensor_tensor(sq, qks[:], qks[:], ALU.mult)
                    ssum = s12.tile([P, 3], f32, tag="ssum")
                    nc.vector.tensor_reduce(ssum, sq[:], mybir.AxisListType.X,
                                            ALU.add)
                    rms = s12.tile([P, 3], f32, tag="rms")
                    nc.scalar.activation(rms, ssum, AF.Sqrt,
                                         bias=eps_sb[:], scale=1.0 / D)
                    rinv = s12.tile([P, 3], f32, tag="rinv")
                    nc.vector.reciprocal_approx_fast(rinv, rms)
                    # k first: its chain feeds gpsimd + the first transpose
                    kn = s12.tile([P, P], bf16, tag="kn")
                    rk_b = rinv[:, 2:3].to_broadcast((P, P))
                    nc.vector.tensor_tensor(kn, qks[:, 2, :], rk_b, ALU.mult)
                    ka = s12.tile([P, P], bf16, tag="ka")
                    nc.gpsimd.tensor_tensor(ka, kn, cosk_sb[:, mm, :], ALU.mult)
                    kb = s12.tile([P, P], bf16, tag="kb")
                    nc.gpsimd.tensor_tensor(kb[:, 0:64], kn[:, 64:128],
                                            sink_sb[:, mm, 0:64], ALU.mult)
                    nc.gpsimd.tensor_tensor(kb[:, 64:128], kn[:, 0:64],
                                            sink_sb[:, mm, 64:128], ALU.mult)
                    rok = s12.tile([P, P], bf16, tag="rok")
                    nc.gpsimd.tensor_tensor(rok, ka, kb, ALU.add)
                    # q-pair: normalize (broadcast rinv) then RoPE, on vector
                    qn = s12.tile([P, 2, P], bf16, tag="qn")
                    rinv_b = rinv[:, 0:2, None].to_broadcast((P, 2, P))
                    nc.vector.tensor_tensor(qn, qks[:, 0:2, :], rinv_b, ALU.mult)
                    cos_b = cosq_sb[:, mm, None, :].to_broadcast((P, 2, D))
                    ro = s12.tile([P, 2, P], bf16, tag="ro")
                    nc.vector.tensor_tensor(ro, qn, cos_b, ALU.mult)
                    rh = s12.tile([P, 2, P], bf16, tag="rh")
                    sinq_mm = sinq_sb[:, mm, :].rearrange("p (x d) -> p x d", x=2)
                    sinA = sinq_mm[:, None, 0, :].to_broadcast((P, 2, 64))
                    sinB = sinq_mm[:, None, 1, :].to_broadcast((P, 2, 64))
                    nc.vector.tensor_tensor(rh[:, :, 0:64], qn[:, :, 64:128],
                                            sinA, ALU.mult)
                    nc.vector.tensor_tensor(rh[:, :, 64:128], qn[:, :, 0:64],
                                            sinB, ALU.mult)
                    nc.vector.tensor_tensor(ro, ro, rh, ALU.add)
                    # transpose to [d, tok] on the PE, deferred into the
                    # middle of the NEXT m's contraction chain
                    col = S * b + P * mm
                    if m >= M - 2:
                        # the tail m-tiles' transposes would otherwise sit in
                        # the PE queue blocking the first attention matmuls;
                        # route them through the DMA XBAR instead (their data
                        # is only needed ~30us later, by the (h0, b1) chunks)
                        nc.sync.dma_start_transpose(kT_sb[:, col:col + P],
                                                    rok[:])
                        nc.sync.dma_start_transpose(qT_sb[:, 0, col:col + P],
                                                    ro[:, 0, :])
                        nc.sync.dma_start_transpose(qT_sb[:, 1, col:col + P],
                                                    ro[:, 1, :])
                    else:
                        tp_pend.extend([
                            (rok[:], kT_sb[:, col:col + P]),
                            (ro[:, 0, :], qT_sb[:, 0, col:col + P]),
                            (ro[:, 1, :], qT_sb[:, 1, col:col + P]),
                        ])

            # prefetch o-projection weights during attention
            wo_sb, _wo_free = tc.tile([P, KO, HS], bf16, name="wo_sb")
            wo_src = woT.ap().rearrange("p (k f) -> p k f", k=KO)
            for k4 in range(0, KO, 4):
                nc.sync.dma_start(wo_sb[:, k4:k4 + 4, :], wo_src[:, k4:k4 + 4, :])

            # ---------------- stage 3: causal attention, head-major so each
            # head's AllToAll overlaps the next head's compute / o-projection
            with tc.tile_pool(name="s3", bufs=4) as s3, \
                 tc.tile_pool(name="s3b", bufs=2) as s3b, \
                 tc.tile_pool(name="ps3", bufs=2, space="PSUM") as ps3:
                pend = []

                def flush_pv(keep=0):
                    # PV/sum for a deferred pair; when it is a chunk's LAST
                    # pair, the chunk finalization (reciprocal, normalize,
                    # store) is emitted here too so every reader is issued
                    # after the matmuls it depends on.
                    while len(pend) > keep:
                        (pq_, h_, b_, c_, pr_, nprs_, o_ps_, sum_ps_) = \
                            pend.pop(0)
                        nc.tensor.matmul(o_ps_, v_sb[:, T * b_ + 2 * pr_, :],
                                         pq_[:, 0, :],
                                         start=(pr_ == 0), stop=False)
                        nc.tensor.matmul(o_ps_, v_sb[:, T * b_ + 2 * pr_ + 1, :],
                                         pq_[:, 1, :],
                                         start=False, stop=(pr_ == nprs_ - 1))
                        ppair = s3.tile([P, CW], bf16, tag="ppair")
                        nc.vector.tensor_tensor(ppair, pq_[:, 0, :],
                                                pq_[:, 1, :], ALU.add)
                        nc.tensor.matmul(sum_ps_, onesq_sb, ppair,
                                         start=(pr_ == 0),
                                         stop=(pr_ == nprs_ - 1))
                        if pr_ == nprs_ - 1:
                            rec = s3b.tile([P, CW], f32, tag="rec")
                            nc.vector.reciprocal_approx_fast(rec, sum_ps_)
                            o_sb = s3b.tile([P, CW], bf16, tag="o_sb")
                            nc.vector.tensor_tensor(o_sb, o_ps_, rec, ALU.mult)
                            r0 = P * (4 * b_ + c_)
                            nc.sync.dma_start(a2a_in[h_][r0:r0 + P, :], o_sb)

                for h in range(NQ):
                    for b in range(2):
                        for c in (3, 2, 1, 0):
                            qv = qT_sb[:, h, S * b + CW * c: S * b + CW * (c + 1)]
                            nprs = (c + 1) * CB // 2
                            o_ps = ps3.tile([P, CW], f32, tag="o")
                            sum_ps = ps3.tile([P, CW], f32, tag="sum")
                            for pr in range(nprs):
                                sp = ps3.tile([P, 2, CW], f32, tag="s")
                                for j in range(2):
                                    kb = 2 * pr + j
                                    nc.tensor.matmul(
                                        sp[:, j, :],
                                        kT_sb[:, S * b + P * kb: S * b + P * (kb + 1)],
                                        qv, start=True, stop=True)
                                # PV lags two pairs behind QK/exp so the PE
                                # never waits on the activation latency
                                flush_pv(keep=1)
                                pq = s3.tile([P, 2, CW], bf16, tag="pT")
                                nc.scalar.activation(pq, sp, AF.Exp, scale=SCL)
                                if 2 * pr >= c * CB:
                                    nc.vector.tensor_tensor(
                                        pq, pq,
                                        masks_sb[:, 2 * pr - c * CB:
                                                 2 * pr - c * CB + 2, :],
                                        ALU.mult)
                                pend.append((pq, h, b, c, pr, nprs,
                                             o_ps, sum_ps))
                    flush_pv()
                    nc.gpsimd.collective_compute(
                        "AllToAll", mybir.AluOpType.bypass,
                        ins=[a2a_in[h][:].opt()], outs=[a2a_out[h][:].opt()],
                        replica_groups=[list(range(N_CORES))],
                    )

            # ---------------- stage 4: output projection for this core's strip
            with tc.tile_pool(name="s4", bufs=1) as s4, \
                 tc.tile_pool(name="s4o", bufs=2) as s4o, \
                 tc.tile_pool(name="ps4", bufs=2, space="PSUM") as ps4:
                attn_sb = []
                for h in range(NQ):
                    # 8 plain contiguous 2D transfers instead of one 1024-way
                    # gather: each row-block of the AllToAll output is already
                    # a [128, CW] tile
                    a_sb = s4.tile([P, KO // NQ, CW], bf16, name=f"attn_sb{h}")
                    for s in range(KO // NQ):
                        nc.sync.dma_start(a_sb[:, s, :],
                                          a2a_out[h][P * s:P * (s + 1), :])
                    attn_sb.append(a_sb)
                accs = {}
                for t in range(CW // P):
                    for oc in range(OCH):
                        ps_o = ps4.tile([P, 512], f32, tag="oproj")
                        for k8 in range(KO // NQ):
                            nc.tensor.matmul(
                                ps_o, attn_sb[0][:, k8, P * t:P * (t + 1)],
                                wo_sb[:, NQ * k8, 512 * oc:512 * (oc + 1)],
                                start=(k8 == 0), stop=(k8 == KO // NQ - 1))
                        acc = s4.tile([P, 512], bf16, tag="acc", bufs=16)
                        nc.scalar.copy(acc, ps_o)
                        accs[(t, oc)] = acc
                for t in range(CW // P):
                    for oc in range(OCH):
                        ps_o = ps4.tile([P, 512], f32, tag="oproj")
                        for k8 in range(KO // NQ):
                            nc.tensor.matmul(
                                ps_o, attn_sb[1][:, k8, P * t:P * (t + 1)],
                                wo_sb[:, NQ * k8 + 1, 512 * oc:512 * (oc + 1)],
                                start=(k8 == 0), stop=(k8 == KO // NQ - 1))
                        osb = s4o.tile([P, 512], f32, tag="osb")
                        nc.vector.tensor_tensor(osb, ps_o, accs[(t, oc)], ALU.add)
                        nc.sync.dma_start(
                            out_d.ap()[P * t:P * (t + 1), 512 * oc:512 * (oc + 1)], osb)
            _wo_free()

    nc.compile()
    return nc


def shard_inputs(inputs, S=2048, HS=2048):
    """Full problem inputs -> list of 8 per-core in_maps (host-side prep)."""
    x = np.asarray(inputs["x"], np.float32)
    cos = np.asarray(inputs["cos"], np.float32)
    sin = np.asarray(inputs["sin"], np.float32)
    wq = np.asarray(inputs["wq"], np.float32)
    wk = np.asarray(inputs["wk"], np.float32)
    wv = np.asarray(inputs["wv"], np.float32)
    wo = np.asarray(inputs["wo"], np.float32)
    qw = np.asarray(inputs["q_norm_w"], np.float32)
    kw = np.asarray(inputs["k_norm_w"], np.float32)

    T = S // P
    M = 2 * T
    CW = S // 4
    CB = CW // P

    KT = HS // P
    xT_t = np.ascontiguousarray(
        x.reshape(M, P, KT, P).transpose(0, 3, 2, 1).reshape(M, P, HS)).astype(BF16)

    sgn = np.concatenate([-np.ones(64, np.float32), np.ones(64, np.float32)])

    def tile_p(a):
        # [(n*P), inner] row-major -> [P, n*inner] partition-major
        n = a.shape[0] // P
        return np.ascontiguousarray(
            a.reshape(n, P, a.shape[1]).transpose(1, 0, 2).reshape(P, -1))

    def fold(w):
        w_rot = np.concatenate([w[64:], w[:64]])
        c = tile_p((cos * w[None, :]).astype(np.float32)).astype(BF16)
        sn = tile_p((sin * (w_rot * sgn)[None, :]).astype(np.float32)).astype(BF16)
        return c, sn

    cosq, sinq = fold(qw)
    cosk, sink = fold(kw)

    r = np.arange(P)[:, None]
    t = np.arange(CW)[None, :]
    masks = np.stack([(r <= t - P * j) for j in range(CB)], axis=1).astype(BF16)

    onesq = np.ones((P, P), BF16)
    ident = np.eye(P, dtype=np.float32).astype(BF16)
    woT = tile_p(np.ascontiguousarray(wo.T)).astype(BF16)

    in_maps = []
    for c in range(N_CORES):
        kvh = c // 2
        wq_c = wq[2 * c * D:(2 * c + 2) * D]       # [256, HS]
        wk_c = wk[kvh * D:(kvh + 1) * D]           # [128, HS]
        wv_c = wv[kvh * D:(kvh + 1) * D]           # [128, HS]
        wqkv = np.concatenate([wq_c, wk_c, wv_c], axis=0)  # [512, HS]
        wqkvT = tile_p(np.ascontiguousarray(wqkv.T)).astype(BF16)  # [P, KT*512]
        in_maps.append({
            "xT": xT_t, "wqkvT": wqkvT, "woT": woT,
            "cosq": cosq, "sinq": sinq, "cosk": cosk, "sink": sink,
            "masks": masks, "onesq": onesq, "ident": ident,
        })
    return in_maps


def assemble(outs, S=2048, HS=2048):
    """Per-core strip outputs -> full [B, S, HS] output."""
    CW = S // 4
    full = np.empty((B, S, HS), np.float32)
    for c in range(N_CORES):
        full[c // 4, (c % 4) * CW:(c % 4 + 1) * CW, :] = \
            np.asarray(outs[c]).astype(np.float32)
    return full


_CACHE = {}


def _get_compiled(S=2048, HS=2048):
    key = (S, HS)
    if key not in _CACHE:
        _CACHE[key] = build(S, HS)
    return _CACHE[key]


def _ensure_ntff_hook():
    """The image's antenv lacks axon_hooks; synthesize it so trace=True works."""
    import types
    try:
        from antenv.axon_hooks import get_axon_ntff_profile_hook  # noqa: F401
        return
    except ImportError:
        pass
    import antenv
    from trn_agent_boot.trn_boot import _ntff_profile_via_ctypes
    mod = types.ModuleType("antenv.axon_hooks")
    mod._hook = _ntff_profile_via_ctypes("/opt/axon/libaxon_pjrt.so")
    mod.set_axon_ntff_profile_hook = lambda h: setattr(mod, "_hook", h)
    mod.get_axon_ntff_profile_hook = lambda: mod._hook
    sys.modules["antenv.axon_hooks"] = mod
    antenv.axon_hooks = mod


def run(inputs, S=2048, HS=2048, trace=False, tmpdir=None):
    import concourse.bass_utils as bu
    if trace:
        _ensure_ntff_hook()
        bu.upload_artifacts = lambda d: ""  # no artifact bucket in this container
    nc = _get_compiled(S, HS)
    in_maps = shard_inputs(inputs, S, HS)
    res = bu.run_bass_kernel_spmd(nc, in_maps, core_ids=list(range(N_CORES)),
                                  trace=trace, tmpdir=tmpdir)
    out = assemble([r["out"] for r in res.results], S, HS)
    return out, res.exec_time_ns


def kernel(**inputs):
    out, _ = run(inputs)
    return out
